# revision 16
# baseline (speedup 1.0000x reference)
"""Trainium2 Bass kernel for nn_BertSelfAttention_43404939493966.

BERT self-attention with adaptive per-segment scaling:
  q/k/v = hidden @ W{q,k,v}.T + b        (biases are spec'd zero -> skipped)
  scores = q k^T / 8,  scaled per (batch,row,col) segment rule, softmax, @v

Sharding: 8 cores = 4 batches x 2 head-groups (8 heads each).
Each core gets host-pretransposed bf16 operands:
  xt  = hidden[b].T                     [H=1024, S=1024]
  wqk = paired W(q|k) chunk columns     [4, 1024, 256]
  wvt = Wv[g*512:(g+1)*512].T           [1024, 512]
  wm1 = (w_seg(q) - 1)                  [1, S]
  mkey= 1[key >= idx2]                  [1, S]
and returns ctx^T for its head-group  [512, S] f32.

Device algorithm (per core, one SPMD program):
  Segment scaling is exact via scale(k,q) = 1 + mkey(k)*(w(q)-1):
    scoresT = KT^T.QT + (KT*mkey)^T.(QT*(w-1))
  Both terms are computed in a SINGLE full-width (K=128) matmul by
  stacking per head h the pair [k_h ; k_h*mkey] (kaug) against
  [q_h ; q_h*(w-1)] (qaug) on the partition axis.  The stacked halves
  are built from the projection psums with partition-aligned DVE
  copies, a partition-shifted SBUF->SBUF SWDGE DMA duplicate, and an
  aligned DVE multiply (even heads: raw top/scaled bottom; odd heads
  reversed, matching the psum half each head lands in).
  exp on ScalarE (scale=1/8 folded in), bf16 probs out.  Each scores
  matmul pair is interleaved with two ctx matmuls of the previous head
  (or a V-projection group for the first two heads) so the PE never
  stalls on the exp draining the scores psum.
  ctx^T = V_aug^T @ probsT with V augmented by a ones-column, so the
  softmax denominator falls out of the same matmul (psum row 64);
  normalize with approx-reciprocal + partition-broadcast + multiply,
  lagged one head behind the ctx matmuls.

attention_mask is all-zeros by spec (fill=zeros) and is not applied.
"""

import numpy as np
import ml_dtypes
from contextlib import ExitStack

import concourse.bass as bass
import concourse.tile as tile
from concourse import bacc, mybir
from concourse.bass_utils import run_bass_kernel_spmd

B, S, H = 4, 1024, 1024
NH, HD = 16, 64
NCORES = 8
HG = 512          # head-group width (8 heads x 64)
KC = 8            # 128-wide key chunks
PC = 128

BF16 = mybir.dt.bfloat16
F32 = mybir.dt.float32


def _build_program():
    nc = bacc.Bacc("TRN2", target_bir_lowering=False, debug=False)

    XT = nc.dram_tensor("xt", (H, S), BF16, kind="ExternalInput")
    WQK = nc.dram_tensor("wqk", (4, H, 2 * PC), BF16, kind="ExternalInput")
    WVT = nc.dram_tensor("wvt", (H, HG), BF16, kind="ExternalInput")
    WM1 = nc.dram_tensor("wm1", (1, S), BF16, kind="ExternalInput")
    MKEY = nc.dram_tensor("mkey", (1, S), BF16, kind="ExternalInput")
    OUT = nc.dram_tensor("out_t", (HG, S), F32, kind="ExternalOutput")

    Exp = mybir.ActivationFunctionType.Exp

    with tile.TileContext(nc) as tc:
        with ExitStack() as ctx:
            persist = ctx.enter_context(tc.tile_pool(name="persist", bufs=1))

            # stacked score operands: [:, h, :] is head h's 128-deep
            # contraction tile ([raw;scaled] even h, [scaled;raw] odd h)
            qaug = persist.tile([PC, 8, S], BF16)
            kaug = persist.tile([PC, 8, S], BF16)
            vaug = persist.tile([PC, 8, 8, HD + 1], BF16)  # [p, s-chunk, head, d+1]
            wm1b = persist.tile([PC, S], BF16)
            mkb = persist.tile([PC, S], BF16)

            wrow = persist.tile([1, S], BF16)
            mrow = persist.tile([1, S], BF16)
            nc.sync.dma_start(wrow, WM1[:, :])
            nc.sync.dma_start(mrow, MKEY[:, :])
            nc.gpsimd.partition_broadcast(wm1b, wrow)
            nc.gpsimd.partition_broadcast(mkb, mrow)
            nc.vector.memset(vaug[:, :, :, HD:HD + 1], 1.0)

            # ---------------- pools ----------------
            xw = ctx.enter_context(tc.tile_pool(name="xw", bufs=1))
            pp = ctx.enter_context(tc.tile_pool(name="pp", bufs=2, space="PSUM"))
            sp = ctx.enter_context(tc.tile_pool(name="sp", bufs=2, space="PSUM"))
            cp = ctx.enter_context(tc.tile_pool(name="cp", bufs=2, space="PSUM"))
            probs = ctx.enter_context(tc.tile_pool(name="probs", bufs=3))
            octp = ctx.enter_context(tc.tile_pool(name="octp", bufs=3))
            rcp = ctx.enter_context(tc.tile_pool(name="rcp", bufs=3))
            dupp = ctx.enter_context(tc.tile_pool(name="dupp", bufs=3))

            # consolidated loads: few large DMAs (HW splits each across
            # the 16 SDMA engines).  wq/wk chunk columns are host-paired
            # per m so head pair m is gated on 0.5MB + xt, not 4MB.
            xta = xw.tile([PC, 8, S], BF16, tag="xta", name="xta")
            wqka = xw.tile([PC, 4, 8, 2 * PC], BF16, tag="wqka", name="wqka")
            wva = xw.tile([PC, 8, HG], BF16, tag="wva", name="wva")
            nc.sync.dma_start(
                wqka[:, 0], WQK[0].rearrange("(k p) c -> p k c", p=PC))
            # per-chunk xt loads: proj(0)'s k-loop streams chunk-by-chunk
            # DURING the transfer window instead of gating on a whole
            # 1MB DMA (the per-DMA dispatch is ~0.7us, which pipelines
            # with the ~0.7us per-chunk transfer)
            for k in range(8):
                nc.sync.dma_start(xta[:, k, :], XT[k * PC:(k + 1) * PC, :])
            # wv before the remaining wqk chunks: the V groups are
            # interleaved into scores(0), well before proj_qk(1..3)
            nc.sync.dma_start(wva, WVT[:, :].rearrange("(k p) f -> p k f", p=PC))
            for m in range(1, 4):
                nc.sync.dma_start(
                    wqka[:, m], WQK[m].rearrange("(k p) c -> p k c", p=PC))
            xts = [xta[:, k, :] for k in range(8)]
            wvs = [wva[:, k, :] for k in range(8)]

            def proj_qk(m):
                """Project head pair (2m, 2m+1) and build their stacked
                qaug/kaug tiles.  Psum half 0:64 is head 2m, 64:128 is
                head 2m+1; the other (scaled) half of each aug tile is a
                DMA partition-dup followed by an aligned DVE multiply."""
                h0, h1 = 2 * m, 2 * m + 1
                for wi, aug, brd, t in ((0, qaug, wm1b, "q"),
                                        (1, kaug, mkb, "k")):
                    for n in range(2):
                        ps = pp.tile([PC, 512], F32, tag="ppsum",
                                     name=f"ppsum_{t}_{m}_{n}")
                        for k in range(8):
                            nc.tensor.matmul(
                                ps,
                                lhsT=wqka[:, m, k, wi * PC:(wi + 1) * PC],
                                rhs=xts[k][:, n * 512:(n + 1) * 512],
                                start=(k == 0), stop=(k == 7),
                            )
                        qs = slice(n * 512, (n + 1) * 512)
                        nc.vector.tensor_copy(aug[0:HD, h0, qs], ps[0:HD, :])
                        nc.vector.tensor_copy(aug[HD:PC, h1, qs], ps[HD:PC, :])
                    dup = dupp.tile([PC, S], BF16, tag="dup",
                                    name=f"dup_{t}_{m}", bufs=3)
                    # SWDGE ring: keeps these latency-critical partition
                    # dups off the HWDGE ring that carries the big loads
                    nc.gpsimd.dma_start(dup[HD:PC, :], aug[0:HD, h0, :])
                    nc.gpsimd.dma_start(dup[0:HD, :], aug[HD:PC, h1, :])
                    nc.vector.tensor_mul(aug[HD:PC, h0, :], dup[HD:PC, :],
                                         brd[HD:PC, :])
                    nc.vector.tensor_mul(aug[0:HD, h1, :], dup[0:HD, :],
                                         brd[0:HD, :])

            def proj_v_group(sc):
                """One V s-chunk accumulation group (8 matmuls)."""
                ps = pp.tile([PC, 512], F32, tag="ppsum", name=f"vpsum_{sc}")
                for k in range(8):
                    nc.tensor.matmul(
                        ps,
                        lhsT=xts[k][:, sc * PC:(sc + 1) * PC],
                        rhs=wvs[k][:, :],
                        start=(k == 0), stop=(k == 7),
                    )
                nc.vector.tensor_copy(
                    vaug[:, sc, :, 0:HD],
                    ps.rearrange("p (h d) -> p h d", h=8),
                )

            def scores_pair(h, pt, kc):
                """The stacked scores matmul pair + exp for one key chunk."""
                psc = sp.tile([PC, S], F32, tag="spsum",
                              name=f"spsum_{h}_{kc}")
                ks = slice(kc * PC, (kc + 1) * PC)
                for qc in range(2):
                    qs = slice(qc * 512, (qc + 1) * 512)
                    nc.tensor.matmul(
                        psc[:, qs],
                        lhsT=kaug[:, h, ks],
                        rhs=qaug[:, h, qs],
                        start=True, stop=True,
                    )
                nc.scalar.activation(
                    out=pt[:, kc, :], in_=psc[:, :],
                    func=Exp, scale=0.125,
                )

            def ctx_evict(h, cpss):
                """Psum eviction + denominator extraction for head h."""
                parts = []
                for qc, cps in enumerate(cpss):
                    cs = octp.tile([HD + 1, 512], F32, tag="cstage",
                                   name=f"cstage_{h}_{qc}", bufs=4)
                    nc.vector.tensor_copy(cs, cps[:, :])
                    rc = rcp.tile([1, 512], F32, tag="rc",
                                  name=f"rc_{h}_{qc}", bufs=4)
                    nc.sync.dma_start(rc[:, :], cs[HD:HD + 1, :])
                    parts.append((cs, rc))
                return parts

            def ctx_psums(h):
                return [cp.tile([HD + 1, 512], F32, tag="cpsum",
                                name=f"cpsum_{h}_{qc}") for qc in range(2)]

            def scores_v(h, pt, scs):
                """scores(h) with V-projection groups as PE filler
                (one V group per two key chunks)."""
                for kc in range(8):
                    scores_pair(h, pt, kc)
                    if kc % 2 == 1:
                        proj_v_group(scs[kc // 2])

            def scores_ctx(h, pt, hp, ptp):
                """scores(h) interleaved with ctx matmuls of head hp:
                per key chunk one scores pair + the two ctx accumulation
                steps, so the PE outruns the exp draining the scores
                psum and never stalls on it."""
                cpss = ctx_psums(hp)
                for kc in range(8):
                    scores_pair(h, pt, kc)
                    for qc in range(2):
                        nc.tensor.matmul(
                            cpss[qc],
                            lhsT=vaug[:, kc, hp, :],
                            rhs=ptp[:, kc, qc * 512:(qc + 1) * 512],
                            start=(kc == 0), stop=(kc == 7),
                        )
                return ctx_evict(hp, cpss)

            def ctx_mm(h, pt):
                """Un-interleaved ctx for head h (pipeline tail)."""
                cpss = ctx_psums(h)
                for qc in range(2):
                    for kc in range(8):
                        nc.tensor.matmul(
                            cpss[qc],
                            lhsT=vaug[:, kc, h, :],
                            rhs=pt[:, kc, qc * 512:(qc + 1) * 512],
                            start=(kc == 0), stop=(kc == 7),
                        )
                return ctx_evict(h, cpss)

            def ctx_fin(h, parts):
                for qc, (cs, rc) in enumerate(parts):
                    qs = slice(qc * 512, (qc + 1) * 512)
                    rc2 = rcp.tile([1, 512], F32, tag="rc2",
                                   name=f"rc2_{h}_{qc}")
                    # approx reciprocal on DVE (~51 ULP, fine for softmax
                    # denominators).  Exact `reciprocal()` costs 3.3us per
                    # call; ACT Reciprocal lives in a different table set
                    # than Exp and forces a 1.3us table reload per switch.
                    # Denominators are sums of positive exps, so the
                    # approx edge cases (0/denorm/inf) cannot occur.
                    nc.vector.reciprocal_approx_fast(out=rc2[:, :],
                                                     in_=rc[:, :])
                    rb = rcp.tile([HD, 512], F32, tag="rb",
                                  name=f"rb_{h}_{qc}")
                    nc.gpsimd.partition_broadcast(rb, rc2)
                    ot = octp.tile([HD, 512], F32, tag="ot",
                                   name=f"ot_{h}_{qc}")
                    nc.vector.tensor_mul(ot, cs[0:HD, :], rb)
                    nc.sync.dma_start(OUT[h * HD:(h + 1) * HD, qs], ot)

            def pthead(h):
                return probs.tile([PC, KC, S], BF16, tag="probs",
                                  name=f"probs_{h}", bufs=3)

            pts = [None] * 8
            proj_qk(0)
            pts[0] = pthead(0); scores_v(0, pts[0], [0, 1, 2, 3])
            proj_qk(1)
            pts[1] = pthead(1); scores_v(1, pts[1], [4, 5, 6, 7])
            cx0 = ctx_mm(0, pts[0])
            proj_qk(2)
            pts[2] = pthead(2)
            cx1 = scores_ctx(2, pts[2], 1, pts[1])
            ctx_fin(0, cx0)
            proj_qk(3)
            pts[3] = pthead(3)
            cx2 = scores_ctx(3, pts[3], 2, pts[2])
            ctx_fin(1, cx1)
            pts[4] = pthead(4)
            cx3 = scores_ctx(4, pts[4], 3, pts[3])
            ctx_fin(2, cx2)
            pts[5] = pthead(5)
            cx4 = scores_ctx(5, pts[5], 4, pts[4])
            ctx_fin(3, cx3)
            pts[6] = pthead(6)
            cx5 = scores_ctx(6, pts[6], 5, pts[5])
            ctx_fin(4, cx4)
            pts[7] = pthead(7)
            cx6 = scores_ctx(7, pts[7], 6, pts[6])
            ctx_fin(5, cx5)
            cx7 = ctx_mm(7, pts[7])
            ctx_fin(6, cx6)
            ctx_fin(7, cx7)

    nc.compile()
    return nc


_NC_CACHE = None


def _get_program():
    global _NC_CACHE
    if _NC_CACHE is None:
        _NC_CACHE = _build_program()
    return _NC_CACHE


def prep_in_maps(inputs):
    """Host-side shard prep (layout transforms only) -> per-core in_maps."""
    hs = np.asarray(inputs["hidden_states"], dtype=np.float32)
    Wq = np.asarray(inputs["Wq"], dtype=np.float32)
    Wk = np.asarray(inputs["Wk"], dtype=np.float32)
    Wv = np.asarray(inputs["Wv"], dtype=np.float32)
    sep = np.asarray(inputs["sep_idx"])
    w0c = float(np.clip(np.asarray(inputs["w0"], np.float32)[0], 0.0, 0.5))
    w1c = float(np.clip(np.asarray(inputs["w1"], np.float32)[0], 0.5, 1.0))
    idx2 = np.asarray(sep[:, 2], dtype=np.int64)

    bf = ml_dtypes.bfloat16
    pos = np.arange(S)

    xt_b = [np.ascontiguousarray(hs[b].T).astype(bf) for b in range(B)]
    wm1_b = []
    mk_b = []
    for b in range(B):
        wseg = np.where(pos < idx2[b], w0c, w1c).astype(np.float32) - 1.0
        wm1_b.append(wseg.reshape(1, S).astype(bf))
        mk_b.append((pos >= idx2[b]).astype(np.float32).reshape(1, S).astype(bf))
    wqk_g = []
    for g in range(2):
        wqt = Wq[g * HG:(g + 1) * HG, :].T   # [H, HG]
        wkt = Wk[g * HG:(g + 1) * HG, :].T
        paired = np.stack(
            [np.concatenate([wqt[:, m * PC:(m + 1) * PC],
                             wkt[:, m * PC:(m + 1) * PC]], axis=1)
             for m in range(4)], axis=0)     # [4, H, 2*PC]
        wqk_g.append(np.ascontiguousarray(paired).astype(bf))
    wvt_g = [np.ascontiguousarray(Wv[g * HG:(g + 1) * HG, :].T).astype(bf)
             for g in range(2)]

    in_maps = []
    for c in range(NCORES):
        b, g = c % B, c // B
        in_maps.append({
            "xt": xt_b[b],
            "wqk": wqk_g[g],
            "wvt": wvt_g[g],
            "wm1": wm1_b[b],
            "mkey": mk_b[b],
        })
    return in_maps


def kernel(hidden_states, attention_mask, sep_idx, Wq, bq, Wk, bk, Wv, bv,
           w0, w1):
    in_maps = prep_in_maps({
        "hidden_states": hidden_states, "sep_idx": sep_idx,
        "Wq": Wq, "Wk": Wk, "Wv": Wv, "w0": w0, "w1": w1,
    })
    nc = _get_program()
    res = run_bass_kernel_spmd(nc, in_maps, core_ids=list(range(NCORES)))

    out = np.empty((B, S, H), dtype=np.float32)
    for c in range(NCORES):
        b, g = c % B, c // B
        out[b, :, g * HG:(g + 1) * HG] = res.results[c]["out_t"].T
    return out


# revision 21
# speedup vs baseline: 1.0132x; 1.0132x over previous
"""Trainium2 Bass kernel for nn_BertSelfAttention_43404939493966.

BERT self-attention with adaptive per-segment scaling:
  q/k/v = hidden @ W{q,k,v}.T + b        (biases are spec'd zero -> skipped)
  scores = q k^T / 8,  scaled per (batch,row,col) segment rule, softmax, @v

Sharding: 8 cores = 4 batches x 2 head-groups (8 heads each).
Each core gets host-pretransposed bf16 operands:
  xt  = hidden[b].T                     [H=1024, S=1024]
  wqk = paired W(q|k) chunk columns     [4, 1024, 256]
  wvt = Wv[g*512:(g+1)*512].T           [1024, 512]
  wm1 = (w_seg(q) - 1)                  [1, S]
  mkey= 1[key >= idx2]                  [1, S]
and returns ctx^T for its head-group  [512, S] f32.

Device algorithm (per core, one SPMD program):
  Segment scaling is exact via scale(k,q) = 1 + mkey(k)*(w(q)-1):
    scoresT = KT^T.QT + (KT*mkey)^T.(QT*(w-1))
  Both terms are computed in a SINGLE full-width (K=128) matmul by
  stacking per head h the pair [k_h ; k_h*mkey] (kaug) against
  [q_h ; q_h*(w-1)] (qaug) on the partition axis.  The stacked halves
  are built from the projection psums with partition-aligned DVE
  copies, a partition-shifted SBUF->SBUF SWDGE DMA duplicate, and an
  aligned DVE multiply (even heads: raw top/scaled bottom; odd heads
  reversed, matching the psum half each head lands in).
  exp on ScalarE (scale=1/8 folded in), bf16 probs out.  Each scores
  matmul pair is interleaved with two ctx matmuls of the previous head
  (or a V-projection group for the first two heads) so the PE never
  stalls on the exp draining the scores psum.
  ctx^T = V_aug^T @ probsT with V augmented by a ones-column, so the
  softmax denominator falls out of the same matmul (psum row 64);
  normalize with approx-reciprocal + partition-broadcast + multiply,
  lagged one head behind the ctx matmuls.

attention_mask is all-zeros by spec (fill=zeros) and is not applied.
"""

import numpy as np
import ml_dtypes
from contextlib import ExitStack

import concourse.bass as bass
import concourse.tile as tile
from concourse import bacc, mybir
from concourse.bass_utils import run_bass_kernel_spmd

B, S, H = 4, 1024, 1024
NH, HD = 16, 64
NCORES = 8
HG = 512          # head-group width (8 heads x 64)
KC = 8            # 128-wide key chunks
PC = 128

BF16 = mybir.dt.bfloat16
F32 = mybir.dt.float32


def _build_program():
    nc = bacc.Bacc("TRN2", target_bir_lowering=False, debug=False)

    # weight layouts are host-pre-swizzled partition-major so every DMA
    # descriptor is a multi-KB contiguous run (256-512B descriptors pay
    # a steep HBM small-descriptor penalty)
    XT = nc.dram_tensor("xt", (H, S), BF16, kind="ExternalInput")
    WQK = nc.dram_tensor("wqk", (4, PC, 8, 2 * PC), BF16, kind="ExternalInput")
    WVT = nc.dram_tensor("wvt", (PC, 8, HG), BF16, kind="ExternalInput")
    WM1 = nc.dram_tensor("wm1", (1, S), BF16, kind="ExternalInput")
    MKEY = nc.dram_tensor("mkey", (1, S), BF16, kind="ExternalInput")
    OUT = nc.dram_tensor("out_t", (HG, S), F32, kind="ExternalOutput")

    Exp = mybir.ActivationFunctionType.Exp

    with tile.TileContext(nc) as tc:
        with ExitStack() as ctx:
            persist = ctx.enter_context(tc.tile_pool(name="persist", bufs=1))

            # stacked score operands: [:, h, :] is head h's 128-deep
            # contraction tile ([raw;scaled] even h, [scaled;raw] odd h)
            qaug = persist.tile([PC, 8, S], BF16)
            kaug = persist.tile([PC, 8, S], BF16)
            vaug = persist.tile([PC, 8, 8, HD + 1], BF16)  # [p, s-chunk, head, d+1]
            wm1b = persist.tile([PC, S], BF16)
            mkb = persist.tile([PC, S], BF16)

            wrow = persist.tile([1, S], BF16)
            mrow = persist.tile([1, S], BF16)
            nc.sync.dma_start(wrow, WM1[:, :])
            nc.sync.dma_start(mrow, MKEY[:, :])
            nc.gpsimd.partition_broadcast(wm1b, wrow)
            nc.gpsimd.partition_broadcast(mkb, mrow)
            nc.vector.memset(vaug[:, :, :, HD:HD + 1], 1.0)

            # ---------------- pools ----------------
            xw = ctx.enter_context(tc.tile_pool(name="xw", bufs=1))
            pp = ctx.enter_context(tc.tile_pool(name="pp", bufs=2, space="PSUM"))
            sp = ctx.enter_context(tc.tile_pool(name="sp", bufs=2, space="PSUM"))
            cp = ctx.enter_context(tc.tile_pool(name="cp", bufs=2, space="PSUM"))
            probs = ctx.enter_context(tc.tile_pool(name="probs", bufs=3))
            octp = ctx.enter_context(tc.tile_pool(name="octp", bufs=3))
            rcp = ctx.enter_context(tc.tile_pool(name="rcp", bufs=3))
            dupp = ctx.enter_context(tc.tile_pool(name="dupp", bufs=3))

            # consolidated loads: few large DMAs (HW splits each across
            # the 16 SDMA engines).  wq/wk chunk columns are host-paired
            # per m so head pair m is gated on 0.5MB + xt, not 4MB.
            xta = xw.tile([PC, 8, S], BF16, tag="xta", name="xta")
            wqka = xw.tile([PC, 4, 8, 2 * PC], BF16, tag="wqka", name="wqka")
            wva = xw.tile([PC, 8, HG], BF16, tag="wva", name="wva")
            nc.sync.dma_start(wqka[:, 0], WQK[0])
            # per-chunk xt loads: proj(0)'s k-loop streams chunk-by-chunk
            # DURING the transfer window instead of gating on a whole
            # 1MB DMA (the per-DMA dispatch is ~0.7us, which pipelines
            # with the ~0.7us per-chunk transfer)
            for k in range(8):
                nc.sync.dma_start(xta[:, k, :], XT[k * PC:(k + 1) * PC, :])
            # wv before the remaining wqk chunks: the V groups are
            # interleaved into scores(0), well before proj_qk(1..3)
            nc.sync.dma_start(wva, WVT[:, :, :])
            for m in range(1, 4):
                nc.sync.dma_start(wqka[:, m], WQK[m])
            xts = [xta[:, k, :] for k in range(8)]
            wvs = [wva[:, k, :] for k in range(8)]

            def proj_qk(m):
                """Project head pair (2m, 2m+1) and build their stacked
                qaug/kaug tiles.  Psum half 0:64 is head 2m, 64:128 is
                head 2m+1; the other (scaled) half of each aug tile is a
                DMA partition-dup followed by an aligned DVE multiply."""
                h0, h1 = 2 * m, 2 * m + 1
                for wi, aug, brd, t in ((0, qaug, wm1b, "q"),
                                        (1, kaug, mkb, "k")):
                    for n in range(2):
                        ps = pp.tile([PC, 512], F32, tag="ppsum",
                                     name=f"ppsum_{t}_{m}_{n}")
                        for k in range(8):
                            nc.tensor.matmul(
                                ps,
                                lhsT=wqka[:, m, k, wi * PC:(wi + 1) * PC],
                                rhs=xts[k][:, n * 512:(n + 1) * 512],
                                start=(k == 0), stop=(k == 7),
                            )
                        qs = slice(n * 512, (n + 1) * 512)
                        nc.vector.tensor_copy(aug[0:HD, h0, qs], ps[0:HD, :])
                        nc.vector.tensor_copy(aug[HD:PC, h1, qs], ps[HD:PC, :])
                    dup = dupp.tile([PC, S], BF16, tag="dup",
                                    name=f"dup_{t}_{m}", bufs=3)
                    # SWDGE ring: keeps these latency-critical partition
                    # dups off the HWDGE ring that carries the big loads
                    nc.gpsimd.dma_start(dup[HD:PC, :], aug[0:HD, h0, :])
                    nc.gpsimd.dma_start(dup[0:HD, :], aug[HD:PC, h1, :])
                    nc.vector.tensor_mul(aug[HD:PC, h0, :], dup[HD:PC, :],
                                         brd[HD:PC, :])
                    nc.vector.tensor_mul(aug[0:HD, h1, :], dup[0:HD, :],
                                         brd[0:HD, :])

            def proj_v_group(sc):
                """One V s-chunk accumulation group (8 matmuls)."""
                ps = pp.tile([PC, 512], F32, tag="ppsum", name=f"vpsum_{sc}")
                for k in range(8):
                    nc.tensor.matmul(
                        ps,
                        lhsT=xts[k][:, sc * PC:(sc + 1) * PC],
                        rhs=wvs[k][:, :],
                        start=(k == 0), stop=(k == 7),
                    )
                nc.vector.tensor_copy(
                    vaug[:, sc, :, 0:HD],
                    ps.rearrange("p (h d) -> p h d", h=8),
                )

            def scores_pair(h, pt, kc):
                """The stacked scores matmul pair + exp for one key chunk."""
                psc = sp.tile([PC, S], F32, tag="spsum",
                              name=f"spsum_{h}_{kc}")
                ks = slice(kc * PC, (kc + 1) * PC)
                for qc in range(2):
                    qs = slice(qc * 512, (qc + 1) * 512)
                    nc.tensor.matmul(
                        psc[:, qs],
                        lhsT=kaug[:, h, ks],
                        rhs=qaug[:, h, qs],
                        start=True, stop=True,
                    )
                nc.scalar.activation(
                    out=pt[:, kc, :], in_=psc[:, :],
                    func=Exp, scale=0.125,
                )

            def ctx_evict(h, cpss, qc0=0):
                """Psum eviction + denominator extraction for head h."""
                parts = []
                for qi, cps in enumerate(cpss):
                    qc = qc0 + qi
                    cs = octp.tile([HD + 1, 512], F32, tag="cstage",
                                   name=f"cstage_{h}_{qc}", bufs=4)
                    nc.vector.tensor_copy(cs, cps[:, :])
                    rc = rcp.tile([1, 512], F32, tag="rc",
                                  name=f"rc_{h}_{qc}", bufs=4)
                    nc.sync.dma_start(rc[:, :], cs[HD:HD + 1, :])
                    parts.append((cs, rc))
                return parts

            def ctx_psums(h):
                return [cp.tile([HD + 1, 512], F32, tag="cpsum",
                                name=f"cpsum_{h}_{qc}") for qc in range(2)]

            def scores_v(h, pt, scs):
                """scores(h) with V-projection groups as PE filler
                (one V group per two key chunks)."""
                for kc in range(8):
                    scores_pair(h, pt, kc)
                    if kc % 2 == 1:
                        proj_v_group(scs[kc // 2])

            def scores_ctx(h, pt, hp, ptp):
                """scores(h) interleaved with ctx matmuls of head hp:
                per key chunk one scores pair + the two ctx accumulation
                steps, so the PE outruns the exp draining the scores
                psum and never stalls on it."""
                cpss = ctx_psums(hp)
                for kc in range(8):
                    scores_pair(h, pt, kc)
                    for qc in range(2):
                        nc.tensor.matmul(
                            cpss[qc],
                            lhsT=vaug[:, kc, hp, :],
                            rhs=ptp[:, kc, qc * 512:(qc + 1) * 512],
                            start=(kc == 0), stop=(kc == 7),
                        )
                return ctx_evict(hp, cpss)

            def ctx_mm(h, pt):
                """Un-interleaved ctx for head h (pipeline tail); each
                psum is evicted as soon as its accumulation finishes so
                the qc0 normalize chain overlaps the qc1 matmuls."""
                parts = []
                cpss = ctx_psums(h)
                for qc in range(2):
                    for kc in range(8):
                        nc.tensor.matmul(
                            cpss[qc],
                            lhsT=vaug[:, kc, h, :],
                            rhs=pt[:, kc, qc * 512:(qc + 1) * 512],
                            start=(kc == 0), stop=(kc == 7),
                        )
                    parts += ctx_evict(h, [cpss[qc]], qc0=qc)
                return parts

            def ctx_fin(h, parts):
                for qc, (cs, rc) in enumerate(parts):
                    qs = slice(qc * 512, (qc + 1) * 512)
                    rc2 = rcp.tile([1, 512], F32, tag="rc2",
                                   name=f"rc2_{h}_{qc}")
                    # approx reciprocal on DVE (~51 ULP, fine for softmax
                    # denominators).  Exact `reciprocal()` costs 3.3us per
                    # call; ACT Reciprocal lives in a different table set
                    # than Exp and forces a 1.3us table reload per switch.
                    # Denominators are sums of positive exps, so the
                    # approx edge cases (0/denorm/inf) cannot occur.
                    nc.vector.reciprocal_approx_fast(out=rc2[:, :],
                                                     in_=rc[:, :])
                    rb = rcp.tile([HD, 512], F32, tag="rb",
                                  name=f"rb_{h}_{qc}")
                    nc.gpsimd.partition_broadcast(rb, rc2)
                    ot = octp.tile([HD, 512], F32, tag="ot",
                                   name=f"ot_{h}_{qc}")
                    nc.vector.tensor_mul(ot, cs[0:HD, :], rb)
                    nc.sync.dma_start(OUT[h * HD:(h + 1) * HD, qs], ot)

            def pthead(h):
                return probs.tile([PC, KC, S], BF16, tag="probs",
                                  name=f"probs_{h}", bufs=3)

            pts = [None] * 8
            proj_qk(0)
            pts[0] = pthead(0); scores_v(0, pts[0], [0, 1, 2, 3])
            proj_qk(1)
            pts[1] = pthead(1); scores_v(1, pts[1], [4, 5, 6, 7])
            cx0 = ctx_mm(0, pts[0])
            proj_qk(2)
            pts[2] = pthead(2)
            cx1 = scores_ctx(2, pts[2], 1, pts[1])
            ctx_fin(0, cx0)
            proj_qk(3)
            pts[3] = pthead(3)
            cx2 = scores_ctx(3, pts[3], 2, pts[2])
            ctx_fin(1, cx1)
            pts[4] = pthead(4)
            cx3 = scores_ctx(4, pts[4], 3, pts[3])
            ctx_fin(2, cx2)
            pts[5] = pthead(5)
            cx4 = scores_ctx(5, pts[5], 4, pts[4])
            ctx_fin(3, cx3)
            pts[6] = pthead(6)
            cx5 = scores_ctx(6, pts[6], 5, pts[5])
            ctx_fin(4, cx4)
            pts[7] = pthead(7)
            cx6 = scores_ctx(7, pts[7], 6, pts[6])
            ctx_fin(5, cx5)
            cx7 = ctx_mm(7, pts[7])
            ctx_fin(6, cx6)
            ctx_fin(7, cx7)

    nc.compile()
    return nc


_NC_CACHE = None


def _get_program():
    global _NC_CACHE
    if _NC_CACHE is None:
        _NC_CACHE = _build_program()
    return _NC_CACHE


def prep_in_maps(inputs):
    """Host-side shard prep (layout transforms only) -> per-core in_maps."""
    hs = np.asarray(inputs["hidden_states"], dtype=np.float32)
    Wq = np.asarray(inputs["Wq"], dtype=np.float32)
    Wk = np.asarray(inputs["Wk"], dtype=np.float32)
    Wv = np.asarray(inputs["Wv"], dtype=np.float32)
    sep = np.asarray(inputs["sep_idx"])
    w0c = float(np.clip(np.asarray(inputs["w0"], np.float32)[0], 0.0, 0.5))
    w1c = float(np.clip(np.asarray(inputs["w1"], np.float32)[0], 0.5, 1.0))
    idx2 = np.asarray(sep[:, 2], dtype=np.int64)

    bf = ml_dtypes.bfloat16
    pos = np.arange(S)

    xt_b = [np.ascontiguousarray(hs[b].T).astype(bf) for b in range(B)]
    wm1_b = []
    mk_b = []
    for b in range(B):
        wseg = np.where(pos < idx2[b], w0c, w1c).astype(np.float32) - 1.0
        wm1_b.append(wseg.reshape(1, S).astype(bf))
        mk_b.append((pos >= idx2[b]).astype(np.float32).reshape(1, S).astype(bf))
    wqk_g = []
    for g in range(2):
        wqt = Wq[g * HG:(g + 1) * HG, :].T   # [H, HG]
        wkt = Wk[g * HG:(g + 1) * HG, :].T
        paired = np.stack(
            [np.concatenate([wqt[:, m * PC:(m + 1) * PC],
                             wkt[:, m * PC:(m + 1) * PC]], axis=1)
             for m in range(4)], axis=0)     # [4, H, 2*PC]
        # partition-major swizzle: [4, H=(k p), c] -> [4, p, k, c] so
        # each DMA descriptor is a 4KB contiguous per-partition run
        paired = paired.reshape(4, 8, PC, 2 * PC).transpose(0, 2, 1, 3)
        wqk_g.append(np.ascontiguousarray(paired).astype(bf))
    wvt_g = []
    for g in range(2):
        wvt = Wv[g * HG:(g + 1) * HG, :].T   # [H, HG]
        wvt = wvt.reshape(8, PC, HG).transpose(1, 0, 2)  # [p, k, f]
        wvt_g.append(np.ascontiguousarray(wvt).astype(bf))

    in_maps = []
    for c in range(NCORES):
        b, g = c % B, c // B
        in_maps.append({
            "xt": xt_b[b],
            "wqk": wqk_g[g],
            "wvt": wvt_g[g],
            "wm1": wm1_b[b],
            "mkey": mk_b[b],
        })
    return in_maps


def kernel(hidden_states, attention_mask, sep_idx, Wq, bq, Wk, bk, Wv, bv,
           w0, w1):
    in_maps = prep_in_maps({
        "hidden_states": hidden_states, "sep_idx": sep_idx,
        "Wq": Wq, "Wk": Wk, "Wv": Wv, "w0": w0, "w1": w1,
    })
    nc = _get_program()
    res = run_bass_kernel_spmd(nc, in_maps, core_ids=list(range(NCORES)))

    out = np.empty((B, S, H), dtype=np.float32)
    for c in range(NCORES):
        b, g = c % B, c // B
        out[b, :, g * HG:(g + 1) * HG] = res.results[c]["out_t"].T
    return out


# revision 28
# speedup vs baseline: 1.0306x; 1.0172x over previous
"""Trainium2 Bass kernel for nn_BertSelfAttention_43404939493966.

BERT self-attention with adaptive per-segment scaling:
  q/k/v = hidden @ W{q,k,v}.T + b        (biases are spec'd zero -> skipped)
  scores = q k^T / 8,  scaled per (batch,row,col) segment rule, softmax, @v

Sharding: 8 cores = 4 batches x 2 head-groups (8 heads each).
Each core gets host-pretransposed bf16 operands:
  xt  = hidden[b].T                     [H=1024, S=1024]
  wqk = paired W(q|k) chunk columns     [4, 1024, 256]
  wvt = Wv[g*512:(g+1)*512].T           [1024, 512]
  wm1 = (w_seg(q) - 1)                  [1, S]
  mkey= 1[key >= idx2]                  [1, S]
and returns ctx^T for its head-group  [512, S] f32.

Device algorithm (per core, one SPMD program):
  Segment scaling is exact via scale(k,q) = 1 + mkey(k)*(w(q)-1):
    scoresT = KT^T.QT + (KT*mkey)^T.(QT*(w-1))
  Both terms are computed in a SINGLE full-width (K=128) matmul by
  stacking per head h the pair [k_h ; k_h*mkey] (kaug) against
  [q_h ; q_h*(w-1)] (qaug) on the partition axis.  The stacked halves
  are built from the projection psums with partition-aligned DVE
  copies, a partition-shifted SBUF->SBUF SWDGE DMA duplicate, and an
  aligned DVE multiply (even heads: raw top/scaled bottom; odd heads
  reversed, matching the psum half each head lands in).
  exp on ScalarE (scale=1/8 folded in), bf16 probs out.  Each scores
  matmul pair is interleaved with two ctx matmuls of the previous head
  (or a V-projection group for the first two heads) so the PE never
  stalls on the exp draining the scores psum.
  ctx^T = V_aug^T @ probsT with V augmented by a ones-column, so the
  softmax denominator falls out of the same matmul (psum row 64);
  normalize with approx-reciprocal + partition-broadcast + multiply,
  lagged one head behind the ctx matmuls.

attention_mask is all-zeros by spec (fill=zeros) and is not applied.
"""

import numpy as np
import ml_dtypes
from contextlib import ExitStack

import concourse.bass as bass
import concourse.tile as tile
from concourse import bacc, mybir
from concourse.bass_utils import run_bass_kernel_spmd

B, S, H = 4, 1024, 1024
NH, HD = 16, 64
NCORES = 8
HG = 512          # head-group width (8 heads x 64)
KC = 8            # 128-wide key chunks
PC = 128

BF16 = mybir.dt.bfloat16
F32 = mybir.dt.float32


def _build_program():
    nc = bacc.Bacc("TRN2", target_bir_lowering=False, debug=False)

    # weight layouts are host-pre-swizzled partition-major so every DMA
    # descriptor is a multi-KB contiguous run (256-512B descriptors pay
    # a steep HBM small-descriptor penalty)
    XT = nc.dram_tensor("xt", (H, S), BF16, kind="ExternalInput")
    WQK = nc.dram_tensor("wqk", (4, PC, 8, 2 * PC), BF16, kind="ExternalInput")
    WVT = nc.dram_tensor("wvt", (PC, 8, HG), BF16, kind="ExternalInput")
    WM1 = nc.dram_tensor("wm1", (1, S), BF16, kind="ExternalInput")
    MKEY = nc.dram_tensor("mkey", (1, S), BF16, kind="ExternalInput")
    OUT = nc.dram_tensor("out_t", (HG, S), F32, kind="ExternalOutput")

    Exp = mybir.ActivationFunctionType.Exp

    with tile.TileContext(nc) as tc:
        with ExitStack() as ctx:
            persist = ctx.enter_context(tc.tile_pool(name="persist", bufs=1))

            # stacked score operands: [:, h, :] is head h's 128-deep
            # contraction tile ([raw;scaled] even h, [scaled;raw] odd h)
            qaug = persist.tile([PC, 8, S], BF16)
            kaug = persist.tile([PC, 8, S], BF16)
            vaug = persist.tile([PC, 8, 8, HD + 1], BF16)  # [p, s-chunk, head, d+1]
            wm1b = persist.tile([PC, S], BF16)
            mkb = persist.tile([PC, S], BF16)

            # ones column for the tail-head PE reciprocal broadcast
            ones64 = persist.tile([PC, HD], F32)
            nc.vector.memset(ones64, 1.0)

            wrow = persist.tile([1, S], BF16)
            mrow = persist.tile([1, S], BF16)
            nc.sync.dma_start(wrow, WM1[:, :])
            nc.sync.dma_start(mrow, MKEY[:, :])
            nc.gpsimd.partition_broadcast(wm1b, wrow)
            nc.gpsimd.partition_broadcast(mkb, mrow)
            nc.vector.memset(vaug[:, :, :, HD:HD + 1], 1.0)

            # ---------------- pools ----------------
            xw = ctx.enter_context(tc.tile_pool(name="xw", bufs=1))
            pp = ctx.enter_context(tc.tile_pool(name="pp", bufs=2, space="PSUM"))
            sp = ctx.enter_context(tc.tile_pool(name="sp", bufs=2, space="PSUM"))
            cp = ctx.enter_context(tc.tile_pool(name="cp", bufs=2, space="PSUM"))
            probs = ctx.enter_context(tc.tile_pool(name="probs", bufs=3))
            octp = ctx.enter_context(tc.tile_pool(name="octp", bufs=3))
            rcp = ctx.enter_context(tc.tile_pool(name="rcp", bufs=3))
            dupp = ctx.enter_context(tc.tile_pool(name="dupp", bufs=3))

            # consolidated loads: few large DMAs (HW splits each across
            # the 16 SDMA engines).  wq/wk chunk columns are host-paired
            # per m so head pair m is gated on 0.5MB + xt, not 4MB.
            xta = xw.tile([PC, 8, S], BF16, tag="xta", name="xta")
            wqka = xw.tile([PC, 4, 8, 2 * PC], BF16, tag="wqka", name="wqka")
            wva = xw.tile([PC, 8, HG], BF16, tag="wva", name="wva")
            nc.sync.dma_start(wqka[:, 0], WQK[0])
            # per-chunk xt loads: proj(0)'s k-loop streams chunk-by-chunk
            # DURING the transfer window instead of gating on a whole
            # 1MB DMA (the per-DMA dispatch is ~0.7us, which pipelines
            # with the ~0.7us per-chunk transfer)
            for k in range(8):
                nc.sync.dma_start(xta[:, k, :], XT[k * PC:(k + 1) * PC, :])
            # wv before the remaining wqk chunks: the V groups are
            # interleaved into scores(0), well before proj_qk(1..3)
            nc.sync.dma_start(wva, WVT[:, :, :])
            for m in range(1, 4):
                nc.sync.dma_start(wqka[:, m], WQK[m])
            xts = [xta[:, k, :] for k in range(8)]
            wvs = [wva[:, k, :] for k in range(8)]

            def proj_qk(m):
                """Project head pair (2m, 2m+1) and build their stacked
                qaug/kaug tiles.  Psum half 0:64 is head 2m, 64:128 is
                head 2m+1; the other (scaled) half of each aug tile is a
                DMA partition-dup followed by an aligned DVE multiply."""
                h0, h1 = 2 * m, 2 * m + 1
                for wi, aug, brd, t in ((0, qaug, wm1b, "q"),
                                        (1, kaug, mkb, "k")):
                    for n in range(2):
                        ps = pp.tile([PC, 512], F32, tag="ppsum",
                                     name=f"ppsum_{t}_{m}_{n}")
                        for k in range(8):
                            nc.tensor.matmul(
                                ps,
                                lhsT=wqka[:, m, k, wi * PC:(wi + 1) * PC],
                                rhs=xts[k][:, n * 512:(n + 1) * 512],
                                start=(k == 0), stop=(k == 7),
                            )
                        qs = slice(n * 512, (n + 1) * 512)
                        nc.vector.tensor_copy(aug[0:HD, h0, qs], ps[0:HD, :])
                        nc.vector.tensor_copy(aug[HD:PC, h1, qs], ps[HD:PC, :])
                    dup = dupp.tile([PC, S], BF16, tag="dup",
                                    name=f"dup_{t}_{m}", bufs=3)
                    # SWDGE ring: keeps these latency-critical partition
                    # dups off the HWDGE ring that carries the big loads
                    nc.gpsimd.dma_start(dup[HD:PC, :], aug[0:HD, h0, :])
                    nc.gpsimd.dma_start(dup[0:HD, :], aug[HD:PC, h1, :])
                    nc.vector.tensor_mul(aug[HD:PC, h0, :], dup[HD:PC, :],
                                         brd[HD:PC, :])
                    nc.vector.tensor_mul(aug[0:HD, h1, :], dup[0:HD, :],
                                         brd[0:HD, :])

            def proj_v_group(sc):
                """One V s-chunk accumulation group (8 matmuls)."""
                ps = pp.tile([PC, 512], F32, tag="ppsum", name=f"vpsum_{sc}")
                for k in range(8):
                    nc.tensor.matmul(
                        ps,
                        lhsT=xts[k][:, sc * PC:(sc + 1) * PC],
                        rhs=wvs[k][:, :],
                        start=(k == 0), stop=(k == 7),
                    )
                nc.vector.tensor_copy(
                    vaug[:, sc, :, 0:HD],
                    ps.rearrange("p (h d) -> p h d", h=8),
                )

            def scores_pair(h, pt, kc):
                """The stacked scores matmul pair + exp for one key chunk."""
                psc = sp.tile([PC, S], F32, tag="spsum",
                              name=f"spsum_{h}_{kc}")
                ks = slice(kc * PC, (kc + 1) * PC)
                for qc in range(2):
                    qs = slice(qc * 512, (qc + 1) * 512)
                    nc.tensor.matmul(
                        psc[:, qs],
                        lhsT=kaug[:, h, ks],
                        rhs=qaug[:, h, qs],
                        start=True, stop=True,
                    )
                nc.scalar.activation(
                    out=pt[:, kc, :], in_=psc[:, :],
                    func=Exp, scale=0.125,
                )

            def ctx_evict(h, cpss, qc0=0, extract_rc=True):
                """Psum eviction + denominator extraction for head h."""
                parts = []
                for qi, cps in enumerate(cpss):
                    qc = qc0 + qi
                    cs = octp.tile([HD + 1, 512], F32, tag="cstage",
                                   name=f"cstage_{h}_{qc}", bufs=4)
                    nc.vector.tensor_copy(cs, cps[:, :])
                    rc = None
                    if extract_rc:
                        rc = rcp.tile([1, 512], F32, tag="rc",
                                      name=f"rc_{h}_{qc}", bufs=4)
                        nc.sync.dma_start(rc[:, :], cs[HD:HD + 1, :])
                    parts.append((cs, rc))
                return parts

            def ctx_psums(h):
                return [cp.tile([HD + 1, 512], F32, tag="cpsum",
                                name=f"cpsum_{h}_{qc}") for qc in range(2)]

            def scores_v(h, pt, scs):
                """scores(h) with V-projection groups as PE filler
                (one V group per two key chunks)."""
                for kc in range(8):
                    scores_pair(h, pt, kc)
                    if kc % 2 == 1:
                        proj_v_group(scs[kc // 2])

            def scores_ctx(h, pt, hp, ptp):
                """scores(h) interleaved with ctx matmuls of head hp:
                per key chunk one scores pair + the two ctx accumulation
                steps, so the PE outruns the exp draining the scores
                psum and never stalls on it."""
                cpss = ctx_psums(hp)
                for kc in range(8):
                    scores_pair(h, pt, kc)
                    for qc in range(2):
                        nc.tensor.matmul(
                            cpss[qc],
                            lhsT=vaug[:, kc, hp, :],
                            rhs=ptp[:, kc, qc * 512:(qc + 1) * 512],
                            start=(kc == 0), stop=(kc == 7),
                        )
                return ctx_evict(hp, cpss)

            def ctx_mm(h, pt, extract_rc=True):
                """Un-interleaved ctx for head h (pipeline tail); each
                psum is evicted as soon as its accumulation finishes so
                the qc0 normalize chain overlaps the qc1 matmuls."""
                parts = []
                cpss = ctx_psums(h)
                for qc in range(2):
                    for kc in range(8):
                        nc.tensor.matmul(
                            cpss[qc],
                            lhsT=vaug[:, kc, h, :],
                            rhs=pt[:, kc, qc * 512:(qc + 1) * 512],
                            start=(kc == 0), stop=(kc == 7),
                        )
                    parts += ctx_evict(h, [cpss[qc]], qc0=qc,
                                       extract_rc=extract_rc)
                return parts

            def ctx_fin(h, parts):
                for qc, (cs, rc) in enumerate(parts):
                    qs = slice(qc * 512, (qc + 1) * 512)
                    rc2 = rcp.tile([1, 512], F32, tag="rc2",
                                   name=f"rc2_{h}_{qc}")
                    # approx reciprocal on DVE (~51 ULP, fine for softmax
                    # denominators).  Exact `reciprocal()` costs 3.3us per
                    # call; ACT Reciprocal lives in a different table set
                    # than Exp and forces a 1.3us table reload per switch.
                    # Denominators are sums of positive exps, so the
                    # approx edge cases (0/denorm/inf) cannot occur.
                    nc.vector.reciprocal_approx_fast(out=rc2[:, :],
                                                     in_=rc[:, :])
                    rb = rcp.tile([HD, 512], F32, tag="rb",
                                  name=f"rb_{h}_{qc}")
                    nc.gpsimd.partition_broadcast(rb, rc2)
                    ot = octp.tile([HD, 512], F32, tag="ot",
                                   name=f"ot_{h}_{qc}")
                    nc.vector.tensor_mul(ot, cs[0:HD, :], rb)
                    nc.sync.dma_start(OUT[h * HD:(h + 1) * HD, qs], ot)

            def ctx_fin_pe(h, parts):
                """Tail-head normalize: the approx reciprocal runs in
                place on the denominator row (partition 64) and a
                1-deep PE matmul against a ones column broadcasts it to
                partitions 0:64 -- removing the rc DMA hop (~2.5us
                completion latency) and the GpSimd broadcast from the
                final chains.  The PE is idle at the tail, so the two
                tiny matmuls per head are free there."""
                for qc, (cs, _) in enumerate(parts):
                    qs = slice(qc * 512, (qc + 1) * 512)
                    rc2 = rcp.tile([PC, 512], F32, tag="rc2p",
                                   name=f"rc2p_{h}_{qc}")
                    nc.vector.reciprocal_approx_fast(
                        out=rc2[HD:HD + 1, :], in_=cs[HD:HD + 1, :])
                    rbp = pp.tile([HD, 512], F32, tag="ppsum",
                                  name=f"rbp_{h}_{qc}")
                    nc.tensor.matmul(rbp, lhsT=ones64[HD:HD + 1, :],
                                     rhs=rc2[HD:HD + 1, :],
                                     start=True, stop=True)
                    ot = octp.tile([HD, 512], F32, tag="ot",
                                   name=f"ot_{h}_{qc}")
                    nc.vector.tensor_mul(ot, cs[0:HD, :], rbp)
                    nc.sync.dma_start(OUT[h * HD:(h + 1) * HD, qs], ot)

            def pthead(h):
                return probs.tile([PC, KC, S], BF16, tag="probs",
                                  name=f"probs_{h}", bufs=3)

            pts = [None] * 8
            proj_qk(0)
            pts[0] = pthead(0); scores_v(0, pts[0], [0, 1, 2, 3])
            proj_qk(1)
            pts[1] = pthead(1); scores_v(1, pts[1], [4, 5, 6, 7])
            cx0 = ctx_mm(0, pts[0])
            proj_qk(2)
            pts[2] = pthead(2)
            cx1 = scores_ctx(2, pts[2], 1, pts[1])
            ctx_fin(0, cx0)
            proj_qk(3)
            pts[3] = pthead(3)
            cx2 = scores_ctx(3, pts[3], 2, pts[2])
            ctx_fin(1, cx1)
            pts[4] = pthead(4)
            cx3 = scores_ctx(4, pts[4], 3, pts[3])
            ctx_fin(2, cx2)
            pts[5] = pthead(5)
            cx4 = scores_ctx(5, pts[5], 4, pts[4])
            ctx_fin(3, cx3)
            pts[6] = pthead(6)
            cx5 = scores_ctx(6, pts[6], 5, pts[5])
            ctx_fin(4, cx4)
            pts[7] = pthead(7)
            cx6 = scores_ctx(7, pts[7], 6, pts[6])
            ctx_fin(5, cx5)
            cx7 = ctx_mm(7, pts[7])
            ctx_fin(6, cx6)
            ctx_fin(7, cx7)

    nc.compile()
    return nc


_NC_CACHE = None


def _get_program():
    global _NC_CACHE
    if _NC_CACHE is None:
        _NC_CACHE = _build_program()
    return _NC_CACHE


def prep_in_maps(inputs):
    """Host-side shard prep (layout transforms only) -> per-core in_maps."""
    hs = np.asarray(inputs["hidden_states"], dtype=np.float32)
    Wq = np.asarray(inputs["Wq"], dtype=np.float32)
    Wk = np.asarray(inputs["Wk"], dtype=np.float32)
    Wv = np.asarray(inputs["Wv"], dtype=np.float32)
    sep = np.asarray(inputs["sep_idx"])
    w0c = float(np.clip(np.asarray(inputs["w0"], np.float32)[0], 0.0, 0.5))
    w1c = float(np.clip(np.asarray(inputs["w1"], np.float32)[0], 0.5, 1.0))
    idx2 = np.asarray(sep[:, 2], dtype=np.int64)

    bf = ml_dtypes.bfloat16
    pos = np.arange(S)

    xt_b = [np.ascontiguousarray(hs[b].T).astype(bf) for b in range(B)]
    wm1_b = []
    mk_b = []
    for b in range(B):
        wseg = np.where(pos < idx2[b], w0c, w1c).astype(np.float32) - 1.0
        wm1_b.append(wseg.reshape(1, S).astype(bf))
        mk_b.append((pos >= idx2[b]).astype(np.float32).reshape(1, S).astype(bf))
    wqk_g = []
    for g in range(2):
        wqt = Wq[g * HG:(g + 1) * HG, :].T   # [H, HG]
        wkt = Wk[g * HG:(g + 1) * HG, :].T
        paired = np.stack(
            [np.concatenate([wqt[:, m * PC:(m + 1) * PC],
                             wkt[:, m * PC:(m + 1) * PC]], axis=1)
             for m in range(4)], axis=0)     # [4, H, 2*PC]
        # partition-major swizzle: [4, H=(k p), c] -> [4, p, k, c] so
        # each DMA descriptor is a 4KB contiguous per-partition run
        paired = paired.reshape(4, 8, PC, 2 * PC).transpose(0, 2, 1, 3)
        wqk_g.append(np.ascontiguousarray(paired).astype(bf))
    wvt_g = []
    for g in range(2):
        wvt = Wv[g * HG:(g + 1) * HG, :].T   # [H, HG]
        wvt = wvt.reshape(8, PC, HG).transpose(1, 0, 2)  # [p, k, f]
        wvt_g.append(np.ascontiguousarray(wvt).astype(bf))

    in_maps = []
    for c in range(NCORES):
        b, g = c % B, c // B
        in_maps.append({
            "xt": xt_b[b],
            "wqk": wqk_g[g],
            "wvt": wvt_g[g],
            "wm1": wm1_b[b],
            "mkey": mk_b[b],
        })
    return in_maps


def kernel(hidden_states, attention_mask, sep_idx, Wq, bq, Wk, bk, Wv, bv,
           w0, w1):
    in_maps = prep_in_maps({
        "hidden_states": hidden_states, "sep_idx": sep_idx,
        "Wq": Wq, "Wk": Wk, "Wv": Wv, "w0": w0, "w1": w1,
    })
    nc = _get_program()
    res = run_bass_kernel_spmd(nc, in_maps, core_ids=list(range(NCORES)))

    out = np.empty((B, S, H), dtype=np.float32)
    for c in range(NCORES):
        b, g = c % B, c // B
        out[b, :, g * HG:(g + 1) * HG] = res.results[c]["out_t"].T
    return out
